# revision 15
# baseline (speedup 1.0000x reference)
"""Complex coherency loss, distributed over 8 TRN2 NeuronCores.

Data-parallel over batch: core b computes the partial coherency sum for
batch element b; the host sums the 8x128x2 partials and finishes the
mean.

Layout ("parity"): the host reorders each [C=64, L=16384] shard into
[P=128, N=8192] with partition p = 2c + (l % 2) and free n = l // 2, so
free column n covers the position PAIR (2n, 2n+1). Compute group g then
covers the contiguous position range [4096g, 4096(g+1)), which lets the
first half of the windowed tail run while the main loop is streaming.

Per-core pipeline:
  - gpsimd casting DMAs load inputs f32 -> bf16 (unlocks DVE 2x mode)
  - DVE: 4 cross products + 2 squares (bf16), ACT: 2 squares (bf16)
  - PE : per 512-chunk, 8 matmuls against [128, 8] +/-1 selector weights
         channel-reduce everything into one [8, 2048] PSUM tile
         (row r = 2q + parity: q in {ptr, pti, pa, ta})
  - ACT copies PSUM -> SBUF; one contiguous DMA appends to a [8, 8196]
    DRAM staging buffer (stg[2q+par, n] = channel sum of quantity q at
    position l = 2n + par)
  - Tail (x2 chunks of 64 halo partitions): two strided DMAs load the
    even/odd staging planes into [64, 4*68] halo tiles (partition p'
    holds n = 64*(64k+p') + i); 4 shifted adds per parity rebuild the
    k=5 sliding window sum:
      win[2n]   = E[n]+E[n+1]+E[n+2]+O[n]+O[n+1]
      win[2n+1] = O[n]+O[n+1]+O[n+2]+E[n+1]+E[n+2]
    then ratio = sqrt((wr^2+wi^2)/(wa*wt)) per parity, masked at the
    2 invalid trailing pairs, free-axis reduced and DMA'd to out[128,2].
"""

import numpy as np
import ml_dtypes

import concourse.bass as bass
import concourse.bacc as bacc
import concourse.mybir as mybir
import concourse.tile as tile
from concourse.bass_utils import run_bass_kernel_spmd

B, C, L = 8, 64, 16384
K = 5
P = 128
N = (C * L) // P          # 8192 free positions per core view (pairs)
NVALID = L - K + 1        # 16380
GROUPS = 4
FD = N // GROUPS          # 2048 free columns per compute group
LOAD_FD = 2 * FD          # 4096 free columns per casting DMA
CH = 512                  # matmul moving-dim chunk
STG_W = N + 4             # staging row width (4 zero-pad columns)

F32 = mybir.dt.float32
BF16 = mybir.dt.bfloat16

PROFILE = False
TRACE_DIR = None
LAST_RESULT = None


def _selector_weights() -> np.ndarray:
    """Five [128, 8] weight matrices, packed as [128, 40] bf16.

    Matrix w maps a product tensor into PSUM rows 2q+par (par = p % 2):
      w=0: m1,m2 -> rows 0,1 (ptr, +)    w=1: m3 -> rows 2,3 (pti, +)
      w=2: m4    -> rows 2,3 (pti, -)    w=3: s1,s2 -> rows 4,5 (pa, +)
      w=4: s3,s4 -> rows 6,7 (ta, +)
    """
    w = np.zeros((P, 5 * 8), dtype=np.float32)
    p = np.arange(P)
    h = p % 2
    w[p, 0 * 8 + 0 + h] = 1.0
    w[p, 1 * 8 + 2 + h] = 1.0
    w[p, 2 * 8 + 2 + h] = -1.0
    w[p, 3 * 8 + 4 + h] = 1.0
    w[p, 4 * 8 + 6 + h] = 1.0
    return w.astype(ml_dtypes.bfloat16)


def build_nc() -> bacc.Bacc:
    nc = bacc.Bacc("TRN2", target_bir_lowering=False, debug=False)

    pr_d = nc.dram_tensor("pr", [P, N], F32, kind="ExternalInput").ap()
    pi_d = nc.dram_tensor("pi", [P, N], F32, kind="ExternalInput").ap()
    tr_d = nc.dram_tensor("tr", [P, N], F32, kind="ExternalInput").ap()
    ti_d = nc.dram_tensor("ti", [P, N], F32, kind="ExternalInput").ap()
    out_d = nc.dram_tensor("out", [P, 2], F32, kind="ExternalOutput").ap()
    w_d = nc.inline_tensor(_selector_weights(), name="selw").ap()

    with tile.TileContext(nc) as tc:
        with (
            tc.tile_pool(name="consts", bufs=1) as consts,
            tc.tile_pool(name="ins", bufs=2) as ins,
            tc.tile_pool(name="prods", bufs=2) as prods,
            tc.tile_pool(name="drains", bufs=2) as drains,
            tc.tile_pool(name="fin", bufs=1) as fin,
            tc.tile_pool(name="psum", bufs=2, space="PSUM") as psum,
            tc.tile_pool(name="dram", bufs=1, space="DRAM") as dram,
        ):
            w_sb = consts.tile([P, 5 * 8], BF16)
            nc.sync.dma_start(w_sb[:, :], w_d)

            stg = dram.tile([8, STG_W], F32)

            # Zero the staging tail so halo reads past N are defined.
            zeros = consts.tile([1, 8 * (STG_W - N)], F32)
            nc.vector.memset(zeros[:, :], 0.0)
            nc.sync.dma_start(stg[:, N:STG_W], zeros[:, :])

            # Pre-warm the Sqrt activation table so the lazy table load
            # doesn't land on the serial tail.
            warm = consts.tile([P, 1], F32)
            nc.vector.memset(warm[:, :], 1.0)
            nc.scalar.sqrt(warm[:, :], warm[:, :])

            # Validity mask for tail chunk 1: pairs n = 8190, 8191
            # (positions l >= 16380) sit at [p'=63, f=62..63].
            mask_b = consts.tile([64, 64], F32)
            nc.vector.memset(mask_b[:, :], 1.0)
            nc.sync.dma_start(mask_b[63:64, 62:64], zeros[0:1, 0:2])

            mm_plan = [  # (weight idx, product slot, start, stop)
                (0, 0, True, False),   # m1 = pr*tr
                (0, 1, False, False),  # m2 = pi*ti
                (1, 2, False, False),  # m3 = pi*tr
                (2, 3, False, False),  # m4 = pr*ti (negative weights)
                (3, 4, False, False),  # s1 = pr^2
                (3, 5, False, False),  # s2 = pi^2
                (4, 6, False, False),  # s3 = tr^2
                (4, 7, False, True),   # s4 = ti^2
            ]

            def tail_chunk(k):
                """Winsum + ratio + reduce for halo rows [64k, 64k+64)."""
                halos = []
                for par in range(2):
                    h = fin.tile([64, 4 * 68], F32, name=f"halo{k}{par}",
                                 tag=f"halo{par}")
                    # src[p', q, i] = stg[2q+par, 64*(64k+p') + i]
                    src = bass.AP(
                        tensor=stg.tensor,
                        offset=stg.offset + par * STG_W + 64 * 64 * k,
                        ap=[[64, 64], [2 * STG_W, 4], [1, 68]],
                    )
                    nc.sync.dma_start(
                        h.rearrange("p (q i) -> p q i", q=4), src
                    )
                    halos.append(h.rearrange("p (q i) -> p q i", q=4))
                hE, hO = halos

                wins = []
                for par in range(2):
                    w = fin.tile([64, 4 * 64], F32, name=f"win{k}{par}",
                                 tag=f"win{par}")
                    w_r = w.rearrange("p (q f) -> p q f", q=4)
                    if par == 0:
                        # win[2n] = E[n]+E[n+1]+E[n+2]+O[n]+O[n+1]
                        nc.vector.tensor_add(
                            w_r, hE[:, :, 0:64], hE[:, :, 1:65])
                        nc.vector.tensor_add(w_r, w_r, hE[:, :, 2:66])
                        nc.vector.tensor_add(w_r, w_r, hO[:, :, 0:64])
                        nc.vector.tensor_add(w_r, w_r, hO[:, :, 1:65])
                    else:
                        # win[2n+1] = O[n]+O[n+1]+O[n+2]+E[n+1]+E[n+2]
                        nc.vector.tensor_add(
                            w_r, hO[:, :, 0:64], hO[:, :, 1:65])
                        nc.vector.tensor_add(w_r, w_r, hO[:, :, 2:66])
                        nc.vector.tensor_add(w_r, w_r, hE[:, :, 1:65])
                        nc.vector.tensor_add(w_r, w_r, hE[:, :, 2:66])
                    wins.append(w)

                for par, w in enumerate(wins):
                    wr = w[:, 0:64]
                    wi = w[:, 64:128]
                    wa = w[:, 128:192]
                    wt = w[:, 192:256]
                    n2 = fin.tile([64, 64], F32, name=f"n2_{k}{par}",
                                  tag=f"n2_{par}")
                    t2 = fin.tile([64, 64], F32, name=f"t2_{k}{par}",
                                  tag=f"t2_{par}")
                    nc.vector.tensor_mul(n2[:, :], wr, wr)
                    nc.vector.tensor_mul(t2[:, :], wi, wi)
                    nc.vector.tensor_add(n2[:, :], n2[:, :], t2[:, :])
                    d2 = fin.tile([64, 64], F32, name=f"d2_{k}{par}",
                                  tag=f"d2_{par}")
                    nc.vector.tensor_mul(d2[:, :], wa, wt)
                    rd = fin.tile([64, 64], F32, name=f"rd_{k}{par}",
                                  tag=f"rd_{par}")
                    nc.vector.reciprocal(rd[:, :], d2[:, :])
                    nc.vector.tensor_mul(n2[:, :], n2[:, :], rd[:, :])
                    if k == 1:
                        nc.vector.tensor_mul(
                            n2[:, :], n2[:, :], mask_b[:, :])
                    sq = fin.tile([64, 64], F32, name=f"sq{k}{par}",
                                  tag=f"sq{par}")
                    acc = fin.tile([64, 1], F32, name=f"acc{k}{par}",
                                   tag=f"acc{par}")
                    nc.scalar.activation(
                        sq[:, :], n2[:, :],
                        mybir.ActivationFunctionType.Sqrt,
                        accum_out=acc[:, :],
                    )
                    # out via SWDGE: keeps the Sync HWDGE FIFO free for
                    # the second tail chunk's halo loads
                    nc.gpsimd.dma_start(
                        out_d[64 * k:64 * k + 64, par:par + 1], acc[:, :]
                    )

            for g in range(GROUPS):
                sl = slice(g * FD, (g + 1) * FD)
                # paired input tiles (pr|pi), (tr|ti)
                t_p = ins.tile([P, 2 * FD], BF16, name="t_p")
                t_t = ins.tile([P, 2 * FD], BF16, name="t_t")
                if g == 0:
                    # group 0 primes the pipeline via low-latency HWDGE
                    # f32 loads + DVE casts (SWDGE casting DMAs take
                    # ~15us to deliver their first bytes)
                    f_p = ins.tile([P, 2 * FD], F32, name="f_p", bufs=1)
                    f_t = ins.tile([P, 2 * FD], F32, name="f_t", bufs=1)
                    nc.sync.dma_start(f_p[:, 0:FD], pr_d[:, sl])
                    nc.sync.dma_start(f_t[:, 0:FD], tr_d[:, sl])
                    nc.sync.dma_start(f_p[:, FD:2 * FD], pi_d[:, sl])
                    nc.sync.dma_start(f_t[:, FD:2 * FD], ti_d[:, sl])
                    nc.vector.tensor_copy(t_p[:, :], f_p[:, :])
                    nc.vector.tensor_copy(t_t[:, :], f_t[:, :])
                else:
                    # casting SWDGE DMAs (f32 -> bf16 in the datapath)
                    nc.gpsimd.dma_start(t_p[:, 0:FD], pr_d[:, sl])
                    nc.gpsimd.dma_start(t_t[:, 0:FD], tr_d[:, sl])
                    nc.gpsimd.dma_start(t_p[:, FD:2 * FD], pi_d[:, sl])
                    nc.gpsimd.dma_start(t_t[:, FD:2 * FD], ti_d[:, sl])

                # (pi|pr): block-swapped view of t_p
                t_p_sw = bass.AP(
                    tensor=t_p.tensor,
                    offset=t_p.offset + FD,
                    ap=[list(t_p.ap[0]), [-FD, 2], [1, FD]],
                )
                t_p3 = t_p.rearrange("p (b f) -> p b f", b=2)
                t_t3 = t_t.rearrange("p (b f) -> p b f", b=2)

                # m12 = (pr*tr | pi*ti), m34 = (pi*tr | pr*ti)
                m12 = prods.tile([P, 2 * FD], BF16, name="m12")
                m34 = prods.tile([P, 2 * FD], BF16, name="m34")
                nc.vector.tensor_mul(
                    m12.rearrange("p (b f) -> p b f", b=2), t_p3, t_t3)
                nc.vector.tensor_mul(
                    m34.rearrange("p (b f) -> p b f", b=2), t_p_sw, t_t3)

                # sqp = (pr^2 | pi^2), sqt = (tr^2 | ti^2)
                sqp = prods.tile([P, 2 * FD], BF16, name="sqp")
                sqt = prods.tile([P, 2 * FD], BF16, name="sqt")
                nc.scalar.square(sqp[:, :], t_p[:, :])
                nc.scalar.square(sqt[:, :], t_t[:, :])

                prod_slices = [
                    m12[:, 0:FD], m12[:, FD:2 * FD],
                    m34[:, 0:FD], m34[:, FD:2 * FD],
                    sqp[:, 0:FD], sqp[:, FD:2 * FD],
                    sqt[:, 0:FD], sqt[:, FD:2 * FD],
                ]

                ps = psum.tile([8, FD], F32, name="ps")
                for widx, pslot, start, stop in mm_plan:
                    prod = prod_slices[pslot]
                    lhsT = w_sb[:, widx * 8:(widx + 1) * 8]
                    for kk in range(FD // CH):
                        ks = slice(kk * CH, (kk + 1) * CH)
                        nc.tensor.matmul(
                            ps[:, ks], lhsT, prod[:, ks],
                            start=start, stop=stop,
                        )

                # drain PSUM -> SBUF -> staging; the last group drains in
                # two halves so the final staging columns land sooner
                halves = 2 if g == GROUPS - 1 else 1
                hw = FD // halves
                for hh in range(halves):
                    hsl = slice(hh * hw, (hh + 1) * hw)
                    dr = drains.tile([8, hw], F32, name="dr", tag="dr")
                    nc.scalar.activation(
                        dr[:, :], ps[:, hsl],
                        mybir.ActivationFunctionType.Copy,
                    )
                    nc.sync.dma_start(
                        stg[:, g * FD + hh * hw:g * FD + (hh + 1) * hw],
                        dr[:, :],
                    )

                if g == 2:
                    # halo rows [0, 64) read stg columns [0, 4098):
                    # complete once groups 0..2 staged; runs under g3
                    tail_chunk(0)

            tail_chunk(1)

    nc.compile()
    return nc


_NC = None


def _get_nc() -> bacc.Bacc:
    global _NC
    if _NC is None:
        _NC = build_nc()
    return _NC


def _parity_view(x: np.ndarray) -> np.ndarray:
    # [64, 16384] -> [128, 8192] with partition 2c + (l%2), free l//2
    return np.ascontiguousarray(
        x.reshape(C, N, 2).transpose(0, 2, 1).reshape(P, N)
    )


def kernel(pred_real, pred_imag, targ_real, targ_imag, filter_size=5):
    global LAST_RESULT
    assert int(filter_size) == K
    nc = _get_nc()

    pred_real = np.asarray(pred_real, dtype=np.float32)
    pred_imag = np.asarray(pred_imag, dtype=np.float32)
    targ_real = np.asarray(targ_real, dtype=np.float32)
    targ_imag = np.asarray(targ_imag, dtype=np.float32)

    in_maps = []
    for b in range(B):
        in_maps.append({
            "pr": _parity_view(pred_real[b]),
            "pi": _parity_view(pred_imag[b]),
            "tr": _parity_view(targ_real[b]),
            "ti": _parity_view(targ_imag[b]),
        })

    kwargs = {}
    if PROFILE:
        kwargs = dict(trace=True)
        if TRACE_DIR is not None:
            import os
            os.makedirs(TRACE_DIR, exist_ok=True)
            kwargs["tmpdir"] = TRACE_DIR
    res = run_bass_kernel_spmd(nc, in_maps, core_ids=list(range(B)), **kwargs)
    LAST_RESULT = res

    total = 0.0
    for r in res.results:
        total += float(np.asarray(r["out"], dtype=np.float64).sum())
    coh = total / (B * NVALID)
    return np.float32(1.0 - coh)


# revision 16
# speedup vs baseline: 1.2136x; 1.2136x over previous
"""Complex coherency loss, distributed over 8 TRN2 NeuronCores.

Data-parallel over batch: core b computes the partial coherency sum for
batch element b; the host sums the 8x128x2 partials and finishes the
mean.

Layout ("parity"): the host reorders each [C=64, L=16384] shard into
[P=128, N=8192] with partition p = 2c + (l % 2) and free n = l // 2, so
free column n covers the position PAIR (2n, 2n+1). Compute group g then
covers the contiguous position range [4096g, 4096(g+1)), which lets the
first half of the windowed tail run while the main loop is streaming.

Per-core pipeline:
  - gpsimd casting DMAs load inputs f32 -> bf16 (unlocks DVE 2x mode)
  - DVE: 4 cross products + 2 squares (bf16), ACT: 2 squares (bf16)
  - PE : per 512-chunk, 8 matmuls against [128, 8] +/-1 selector weights
         channel-reduce everything into one [8, 2048] PSUM tile
         (row r = 2q + parity: q in {ptr, pti, pa, ta})
  - ACT copies PSUM -> SBUF; one contiguous DMA appends to a [8, 8196]
    DRAM staging buffer (stg[2q+par, n] = channel sum of quantity q at
    position l = 2n + par)
  - Tail (x2 chunks of 64 halo partitions): two strided DMAs load the
    even/odd staging planes into [64, 4*68] halo tiles (partition p'
    holds n = 64*(64k+p') + i); 4 shifted adds per parity rebuild the
    k=5 sliding window sum:
      win[2n]   = E[n]+E[n+1]+E[n+2]+O[n]+O[n+1]
      win[2n+1] = O[n]+O[n+1]+O[n+2]+E[n+1]+E[n+2]
    then ratio = sqrt((wr^2+wi^2)/(wa*wt)) per parity, masked at the
    2 invalid trailing pairs, free-axis reduced and DMA'd to out[128,2].
"""

import numpy as np
import ml_dtypes

import concourse.bass as bass
import concourse.bacc as bacc
import concourse.mybir as mybir
import concourse.tile as tile
from concourse.bass_utils import run_bass_kernel_spmd

B, C, L = 8, 64, 16384
K = 5
P = 128
N = (C * L) // P          # 8192 free positions per core view (pairs)
NVALID = L - K + 1        # 16380
GROUPS = 4
FD = N // GROUPS          # 2048 free columns per compute group
LOAD_FD = 2 * FD          # 4096 free columns per casting DMA
CH = 512                  # matmul moving-dim chunk
STG_W = N + 4             # staging row width (4 zero-pad columns)

F32 = mybir.dt.float32
BF16 = mybir.dt.bfloat16

PROFILE = False
TRACE_DIR = None
LAST_RESULT = None


def _selector_weights() -> np.ndarray:
    """Five [128, 8] weight matrices, packed as [128, 40] bf16.

    Matrix w maps a product tensor into PSUM rows 2q+par (par = p % 2):
      w=0: m1,m2 -> rows 0,1 (ptr, +)    w=1: m3 -> rows 2,3 (pti, +)
      w=2: m4    -> rows 2,3 (pti, -)    w=3: s1,s2 -> rows 4,5 (pa, +)
      w=4: s3,s4 -> rows 6,7 (ta, +)
    """
    w = np.zeros((P, 5 * 8), dtype=np.float32)
    p = np.arange(P)
    h = p % 2
    w[p, 0 * 8 + 0 + h] = 1.0
    w[p, 1 * 8 + 2 + h] = 1.0
    w[p, 2 * 8 + 2 + h] = -1.0
    w[p, 3 * 8 + 4 + h] = 1.0
    w[p, 4 * 8 + 6 + h] = 1.0
    return w.astype(ml_dtypes.bfloat16)


def build_nc() -> bacc.Bacc:
    nc = bacc.Bacc("TRN2", target_bir_lowering=False, debug=False)

    pr_d = nc.dram_tensor("pr", [P, N], F32, kind="ExternalInput").ap()
    pi_d = nc.dram_tensor("pi", [P, N], F32, kind="ExternalInput").ap()
    tr_d = nc.dram_tensor("tr", [P, N], F32, kind="ExternalInput").ap()
    ti_d = nc.dram_tensor("ti", [P, N], F32, kind="ExternalInput").ap()
    out_d = nc.dram_tensor("out", [P, 2], F32, kind="ExternalOutput").ap()
    w_d = nc.inline_tensor(_selector_weights(), name="selw").ap()

    with tile.TileContext(nc) as tc:
        with (
            tc.tile_pool(name="consts", bufs=1) as consts,
            tc.tile_pool(name="ins", bufs=2) as ins,
            tc.tile_pool(name="prods", bufs=2) as prods,
            tc.tile_pool(name="drains", bufs=2) as drains,
            tc.tile_pool(name="fin", bufs=1) as fin,
            tc.tile_pool(name="psum", bufs=2, space="PSUM") as psum,
            tc.tile_pool(name="dram", bufs=1, space="DRAM") as dram,
        ):
            w_sb = consts.tile([P, 5 * 8], BF16)
            nc.sync.dma_start(w_sb[:, :], w_d)

            stg = dram.tile([8, STG_W], F32)

            # Zero the staging tail so halo reads past N are defined.
            zeros = consts.tile([1, 8 * (STG_W - N)], F32)
            nc.vector.memset(zeros[:, :], 0.0)
            nc.sync.dma_start(stg[:, N:STG_W], zeros[:, :])

            # Pre-warm the Sqrt activation table so the lazy table load
            # doesn't land on the serial tail.
            warm = consts.tile([P, 1], F32)
            nc.vector.memset(warm[:, :], 1.0)
            nc.scalar.sqrt(warm[:, :], warm[:, :])

            # Validity mask for tail chunk 1: pairs n = 8190, 8191
            # (positions l >= 16380) sit at [p'=63, f=62..63].
            mask_b = consts.tile([64, 64], F32)
            nc.vector.memset(mask_b[:, :], 1.0)
            nc.sync.dma_start(mask_b[63:64, 62:64], zeros[0:1, 0:2])

            mm_plan = [  # (weight idx, product slot, start, stop)
                (0, 0, True, False),   # m1 = pr*tr
                (0, 1, False, False),  # m2 = pi*ti
                (1, 2, False, False),  # m3 = pi*tr
                (2, 3, False, False),  # m4 = pr*ti (negative weights)
                (3, 4, False, False),  # s1 = pr^2
                (3, 5, False, False),  # s2 = pi^2
                (4, 6, False, False),  # s3 = tr^2
                (4, 7, False, True),   # s4 = ti^2
            ]

            def tail_chunk(k):
                """Winsum + ratio + reduce for halo rows [64k, 64k+64)."""
                halos = []
                for par in range(2):
                    h = fin.tile([64, 4 * 68], F32, name=f"halo{k}{par}",
                                 tag=f"halo{par}")
                    # src[p', q, i] = stg[2q+par, 64*(64k+p') + i]
                    src = bass.AP(
                        tensor=stg.tensor,
                        offset=stg.offset + par * STG_W + 64 * 64 * k,
                        ap=[[64, 64], [2 * STG_W, 4], [1, 68]],
                    )
                    nc.sync.dma_start(
                        h.rearrange("p (q i) -> p q i", q=4), src
                    )
                    halos.append(h.rearrange("p (q i) -> p q i", q=4))
                hE, hO = halos

                wins = []
                for par in range(2):
                    w = fin.tile([64, 4 * 64], F32, name=f"win{k}{par}",
                                 tag=f"win{par}")
                    w_r = w.rearrange("p (q f) -> p q f", q=4)
                    if par == 0:
                        # win[2n] = E[n]+E[n+1]+E[n+2]+O[n]+O[n+1]
                        nc.vector.tensor_add(
                            w_r, hE[:, :, 0:64], hE[:, :, 1:65])
                        nc.vector.tensor_add(w_r, w_r, hE[:, :, 2:66])
                        nc.vector.tensor_add(w_r, w_r, hO[:, :, 0:64])
                        nc.vector.tensor_add(w_r, w_r, hO[:, :, 1:65])
                    else:
                        # win[2n+1] = O[n]+O[n+1]+O[n+2]+E[n+1]+E[n+2]
                        nc.vector.tensor_add(
                            w_r, hO[:, :, 0:64], hO[:, :, 1:65])
                        nc.vector.tensor_add(w_r, w_r, hO[:, :, 2:66])
                        nc.vector.tensor_add(w_r, w_r, hE[:, :, 1:65])
                        nc.vector.tensor_add(w_r, w_r, hE[:, :, 2:66])
                    wins.append(w)

                for par, w in enumerate(wins):
                    wr = w[:, 0:64]
                    wi = w[:, 64:128]
                    wa = w[:, 128:192]
                    wt = w[:, 192:256]
                    n2 = fin.tile([64, 64], F32, name=f"n2_{k}{par}",
                                  tag=f"n2_{par}")
                    t2 = fin.tile([64, 64], F32, name=f"t2_{k}{par}",
                                  tag=f"t2_{par}")
                    nc.vector.tensor_mul(n2[:, :], wr, wr)
                    nc.vector.tensor_mul(t2[:, :], wi, wi)
                    nc.vector.tensor_add(n2[:, :], n2[:, :], t2[:, :])
                    d2 = fin.tile([64, 64], F32, name=f"d2_{k}{par}",
                                  tag=f"d2_{par}")
                    nc.vector.tensor_mul(d2[:, :], wa, wt)
                    rd = fin.tile([64, 64], F32, name=f"rd_{k}{par}",
                                  tag=f"rd_{par}")
                    nc.vector.reciprocal(rd[:, :], d2[:, :])
                    nc.vector.tensor_mul(n2[:, :], n2[:, :], rd[:, :])
                    if k == 1:
                        nc.vector.tensor_mul(
                            n2[:, :], n2[:, :], mask_b[:, :])
                    sq = fin.tile([64, 64], F32, name=f"sq{k}{par}",
                                  tag=f"sq{par}")
                    acc = fin.tile([64, 1], F32, name=f"acc{k}{par}",
                                   tag=f"acc{par}")
                    nc.scalar.activation(
                        sq[:, :], n2[:, :],
                        mybir.ActivationFunctionType.Sqrt,
                        accum_out=acc[:, :],
                    )
                    # out via SWDGE: keeps the Sync HWDGE FIFO free for
                    # the second tail chunk's halo loads
                    nc.gpsimd.dma_start(
                        out_d[64 * k:64 * k + 64, par:par + 1], acc[:, :]
                    )

            # Small first group so first products arrive early (SWDGE
            # casting DMAs have ~3-4us first-byte latency); small last
            # group so the final staging handoff is short.
            group_fds = [512, 1536, 2048, 2048, 1536, 512]
            assert sum(group_fds) == N
            col = 0
            for g, fd in enumerate(group_fds):
                sl = slice(col, col + fd)
                # paired input tiles (pr|pi), (tr|ti); casting SWDGE DMAs
                t_p = ins.tile([P, 2 * fd], BF16, name="t_p", tag="t_p",
                               padded_shape=[P, 2 * FD])
                t_t = ins.tile([P, 2 * fd], BF16, name="t_t", tag="t_t",
                               padded_shape=[P, 2 * FD])
                nc.gpsimd.dma_start(t_p[:, 0:fd], pr_d[:, sl])
                nc.gpsimd.dma_start(t_t[:, 0:fd], tr_d[:, sl])
                nc.gpsimd.dma_start(t_p[:, fd:2 * fd], pi_d[:, sl])
                nc.gpsimd.dma_start(t_t[:, fd:2 * fd], ti_d[:, sl])

                # (pi|pr): block-swapped view of t_p
                t_p_sw = bass.AP(
                    tensor=t_p.tensor,
                    offset=t_p.offset + fd,
                    ap=[list(t_p.ap[0]), [-fd, 2], [1, fd]],
                )
                t_p3 = t_p.rearrange("p (b f) -> p b f", b=2)
                t_t3 = t_t.rearrange("p (b f) -> p b f", b=2)

                # m12 = (pr*tr | pi*ti), m34 = (pi*tr | pr*ti)
                m12 = prods.tile([P, 2 * fd], BF16, name="m12", tag="m12",
                                 padded_shape=[P, 2 * FD])
                m34 = prods.tile([P, 2 * fd], BF16, name="m34", tag="m34",
                                 padded_shape=[P, 2 * FD])
                nc.vector.tensor_mul(
                    m12.rearrange("p (b f) -> p b f", b=2), t_p3, t_t3)
                nc.vector.tensor_mul(
                    m34.rearrange("p (b f) -> p b f", b=2), t_p_sw, t_t3)

                # sqp = (pr^2 | pi^2), sqt = (tr^2 | ti^2)
                sqp = prods.tile([P, 2 * fd], BF16, name="sqp", tag="sqp",
                                 padded_shape=[P, 2 * FD])
                sqt = prods.tile([P, 2 * fd], BF16, name="sqt", tag="sqt",
                                 padded_shape=[P, 2 * FD])
                nc.scalar.square(sqp[:, :], t_p[:, :])
                nc.scalar.square(sqt[:, :], t_t[:, :])

                prod_slices = [
                    m12[:, 0:fd], m12[:, fd:2 * fd],
                    m34[:, 0:fd], m34[:, fd:2 * fd],
                    sqp[:, 0:fd], sqp[:, fd:2 * fd],
                    sqt[:, 0:fd], sqt[:, fd:2 * fd],
                ]

                ps = psum.tile([8, fd], F32, name="ps", tag="ps",
                               padded_shape=[8, FD])
                for widx, pslot, start, stop in mm_plan:
                    prod = prod_slices[pslot]
                    lhsT = w_sb[:, widx * 8:(widx + 1) * 8]
                    for kk in range(fd // CH):
                        ks = slice(kk * CH, (kk + 1) * CH)
                        nc.tensor.matmul(
                            ps[:, ks], lhsT, prod[:, ks],
                            start=start, stop=stop,
                        )

                dr = drains.tile([8, fd], F32, name="dr", tag="dr",
                                 padded_shape=[8, FD])
                nc.scalar.activation(
                    dr[:, :], ps[:, :], mybir.ActivationFunctionType.Copy
                )
                nc.sync.dma_start(stg[:, col:col + fd], dr[:, :])
                col += fd

                if col == 6144:
                    # halo rows [0, 64) read stg columns [0, 4100):
                    # staged by now; overlaps the remaining groups
                    tail_chunk(0)

            tail_chunk(1)

    nc.compile()
    return nc


_NC = None


def _get_nc() -> bacc.Bacc:
    global _NC
    if _NC is None:
        _NC = build_nc()
    return _NC


def _parity_view(x: np.ndarray) -> np.ndarray:
    # [64, 16384] -> [128, 8192] with partition 2c + (l%2), free l//2
    return np.ascontiguousarray(
        x.reshape(C, N, 2).transpose(0, 2, 1).reshape(P, N)
    )


def kernel(pred_real, pred_imag, targ_real, targ_imag, filter_size=5):
    global LAST_RESULT
    assert int(filter_size) == K
    nc = _get_nc()

    pred_real = np.asarray(pred_real, dtype=np.float32)
    pred_imag = np.asarray(pred_imag, dtype=np.float32)
    targ_real = np.asarray(targ_real, dtype=np.float32)
    targ_imag = np.asarray(targ_imag, dtype=np.float32)

    in_maps = []
    for b in range(B):
        in_maps.append({
            "pr": _parity_view(pred_real[b]),
            "pi": _parity_view(pred_imag[b]),
            "tr": _parity_view(targ_real[b]),
            "ti": _parity_view(targ_imag[b]),
        })

    kwargs = {}
    if PROFILE:
        kwargs = dict(trace=True)
        if TRACE_DIR is not None:
            import os
            os.makedirs(TRACE_DIR, exist_ok=True)
            kwargs["tmpdir"] = TRACE_DIR
    res = run_bass_kernel_spmd(nc, in_maps, core_ids=list(range(B)), **kwargs)
    LAST_RESULT = res

    total = 0.0
    for r in res.results:
        total += float(np.asarray(r["out"], dtype=np.float64).sum())
    coh = total / (B * NVALID)
    return np.float32(1.0 - coh)
